# revision 43
# baseline (speedup 1.0000x reference)
"""Causal single-head attention (S=8192, d=64) on 8 Trainium2 NeuronCores.

Strategy (sequence-parallel, load-balanced over the causal triangle):
  - Split the sequence into 16 chunks of 512 rows. Core m owns query chunks
    A=m and B=15-m, so every core sees (m+1) + (16-m) = 17 (kv-block, q-chunk)
    pairs of 512x512 scores -- perfectly balanced.
  - Each pair is one "slot". The SPMD program is identical on all cores; the
    per-core schedule is baked into the *input data*: the host sends, per
    slot, the kv rows (transposed + bias-augmented) AND the matching
    projected-query panel qtil = (wq_aug wk_aug^T) @ xq_aug^T, so the device
    runs zero per-slot projection matmuls for the score side.
  - Scores are computed transposed, sT[j, i] (kv j on partitions):
      sT = xkvT[:, js]^T @ qtil   (stationary straight from the DMA)
    so the softmax denominator comes from an extra all-ones column of the
    augmented V in the PV matmul, and P^T feeds the PV matmul directly.
  - All matmul operands are bf16 (the correctness gate is 2e-2; bf16 puts
    global rel err ~1e-3): 4x faster v-projection matmuls, half the DMA and
    SBUF traffic, and bf16 pt/v_sb feed PE at 1 cycle/row.
  - The causal mask is needed only on the two diagonal slots (slot 0 and 1)
    and is applied as a post-exp affine_select (zero the j > i half).
  - Per-slot partials accumulate into per-chunk SBUF accumulators via a 0/1
    multiplier input (gamma), keeping the program fully static.
  - PV(t) is emitted AFTER scores(t+1) so the in-order PE stream always has
    exp-independent work while the Act engine (the busiest engine, ~35us of
    exp) drains; steady state is Act-bound.

Math per slot t with kv block rows Xk = x[512b:512b+512], q rows Xq:
  xkvT = [Xk^T; 1]  (65 x 512)     qtil = M2 @ [Xq^T; 1]  (65 x 512, host)
  M2   = wq_aug @ wk_aug^T / 8     (65 x 65, host-precomputed from weights)
  sT   = xkvT[:, js]^T @ qtil      (128 x 512 per 128-row j-subchunk, PSUM)
  pT   = exp(sT)                   (masked to causal on diagonal slots)
  v    = xkvT[:, js]^T @ wv_aug    (128 x 65 per j-subchunk; col 64 = ones)
  oT  += v[js]^T @ pT              (65 x 512; row 64 = softmax denominator)
Final: out[i, :] = oT[0:64, i] / oT[64, i], transposed back via PE transpose.
"""

import sys

sys.path.insert(0, "/opt/trn_rl_repo")

import numpy as np
import concourse.bass as bass
import concourse.mybir as mybir
from concourse import tile
from concourse.bass_utils import run_bass_kernel_spmd

OVERLAP_FINALS = True
N_CORES = 8
S = 8192
D = 64
CH = 512
NCH = S // CH          # 16 chunks
NSLOT = 17             # (kv, q) pairs per core
JS = 128               # j-subchunk (PSUM partition dim)
NJS = CH // JS         # 4
DA = D + 1             # bias-augmented contraction dim

F32 = mybir.dt.float32
BF16 = mybir.dt.bfloat16


def _split_multiwait(nc, max_waits=1):
    """The walrus build in this container accepts only one sync-wait per
    instruction; hoist extra waits onto preceding same-engine NOPs."""
    for func in nc.m.functions:
        for bb in func.blocks:
            new_insts = []
            for inst in bb.instructions:
                si = inst.sync_info
                if si is not None and si.on_wait and len(si.on_wait) > max_waits:
                    waits = list(si.on_wait)
                    rest, head = waits[:-max_waits], waits[-max_waits:]
                    for j, w in enumerate(rest):
                        nop = mybir.InstNoOp(
                            name=f"{inst.name}-wsplit{j}", ins=[], outs=[]
                        )
                        nop.engine = inst.engine
                        nop.sync_info = mybir.SyncInfo(on_wait=[w], on_update=[])
                        new_insts.append(nop)
                    inst.sync_info = mybir.SyncInfo(
                        on_wait=head, on_update=si.on_update
                    )
                new_insts.append(inst)
            bb.instructions = new_insts


def _schedule(m):
    """Slot list [(kv_block, q_chunk)] for core m; diagonal pairs first."""
    A, B = m, NCH - 1 - m
    slots = [(A, A), (B, B)]
    slots += [(b, A) for b in range(A)]
    slots += [(b, B) for b in range(B)]
    gam = [1.0, 0.0] + [1.0] * A + [0.0] * B
    assert len(slots) == NSLOT
    return slots, gam


def _build_program(repeat=1, dynamic=False):
    nc = bass.Bass()

    # xall[:, t, 0, :] = slot t's kv rows transposed+augmented (bf16);
    # [:, t, 1, :] = its host-projected qtil panel. One DMA per slot.
    xall_d = nc.declare_dram_parameter(
        "xall", [DA, NSLOT * 2 * CH], BF16, isOutput=False
    )
    # vall[:, t, s, :] = slot t's host-projected v_aug rows for j-subchunk s
    # (col 64 = ones -> softmax denominator row of the PV output)
    vall_d = nc.declare_dram_parameter(
        "vall", [JS, NSLOT * NJS * DA], BF16, isOutput=False
    )
    # packed fp32 constants: [gamma | identity]
    CPW = NSLOT + DA
    cpack_d = nc.declare_dram_parameter("cpack", [DA, CPW], F32, isOutput=False)
    out_d = nc.declare_dram_parameter("out_pair", [2, CH, D], F32, isOutput=True)

    NXBUF = 6
    with tile.TileContext(nc) as tc:
        with (
            tc.tile_pool(name="consts", bufs=1) as consts,
            tc.tile_pool(name="slot_in", bufs=NXBUF) as slot_in,
            tc.tile_pool(name="slot_mid", bufs=6) as slot_mid,
            tc.tile_pool(name="acc", bufs=1) as accp,
            tc.tile_pool(name="pt", bufs=5) as ptp,
            tc.tile_pool(name="gd", bufs=6) as gdp,
            tc.tile_pool(name="fin", bufs=4) as finp,
            tc.tile_pool(name="ps_s", bufs=2, space="PSUM") as ps_s_p,
            tc.tile_pool(name="ps_o", bufs=2, space="PSUM") as ps_o_p,
        ):
            # ---- x tiles are [128, 2, CH] with only rows 0..64 DMA'd per
            # slot. Rows 65..127 are zeroed ONCE per ring buffer here (and
            # never written again), so the scores matmuls can run with a
            # zero-padded 128-row contraction: the PE streams moving rows
            # at 1 row/cycle only at full 128-partition contraction
            # (measured ~0.41 ns/row vs ~0.83 at 65) -- the padded zeros
            # buy a ~1.76x faster scores stage. The padded qt rows must
            # also be finite (0 * garbage = NaN if garbage is inf/nan), so
            # both halves get the one-time memset.
            xbufs = []
            for b in range(NXBUF):
                x_b = slot_in.tile([JS, 2, CH], BF16, tag="x_t")
                # partition starts must be 0/32/64/96-aligned: zero rows
                # 64..127 (the DMA rewrites row 64 -- the aug row -- each
                # slot; rows 65..127 stay zero for the padded contraction)
                nc.gpsimd.memset(x_b[D:JS, :, :], 0.0)
                xbufs.append(x_b)
            # first slot's x DMA heads the critical path (scores of slot 0)
            x_t0 = xbufs[0]
            nc.sync.dma_start(
                out=x_t0[0:DA, :, :],
                in_=xall_d[:, 0:2 * CH].rearrange("p (h c) -> p h c", h=2),
            )
            cpack = consts.tile([DA, CPW], F32)
            nc.sync.dma_start(out=cpack[:], in_=cpack_d[:])
            gam = cpack[:, 0:NSLOT]
            ident = cpack[:, NSLOT:CPW]

            # ---- body (repeat>1 builds a timing-calibration NEFF) ----
            if dynamic and repeat > 1:
                with tc.For_i(0, repeat, 1):
                    _body(nc, tc, xbufs, slot_mid, ptp, gdp, finp,
                          ps_s_p, ps_o_p, accp,
                          xall_d, vall_d, out_d, gam, ident, False)
            else:
                for _rep in range(repeat):
                    _body(nc, tc, xbufs, slot_mid, ptp, gdp, finp,
                          ps_s_p, ps_o_p, accp,
                          xall_d, vall_d, out_d, gam, ident, _rep == 0)

    _split_multiwait(nc)
    return nc


def _body(nc, tc, xbufs, slot_mid, ptp, gdp, finp,
          ps_s_p, ps_o_p, accp,
          xall_d, vall_d, out_d, gam, ident, x0_ready):
    # ---- per-chunk accumulators ----
    accA = accp.tile([DA, CH], F32, tag="accA")
    accB = accp.tile([DA, CH], F32, tag="accB")

    def dma_x(t):
        # manual ring over the memset-prepped buffers: rows 65..127 stay
        # zero (the scores matmuls' padded contraction), DMA only 0..64
        x_t = xbufs[t % len(xbufs)]
        nc.sync.dma_start(
            out=x_t[0:DA, :, :],
            in_=xall_d[:, t * 2 * CH:(t + 1) * 2 * CH].rearrange(
                "p (h c) -> p h c", h=2
            ),
        )
        return x_t

    def dma_v(t):
        v_sb = slot_mid.tile([JS, NJS * DA], BF16, tag="v_sb")
        nc.sync.dma_start(
            out=v_sb[:], in_=vall_d[:, t * NJS * DA:(t + 1) * NJS * DA]
        )
        return v_sb

    # ---- subchunk-stream score/exp tiling (hybrid) ----
    # The j-subchunks form one stream k = 4t+s (68 total). The two
    # DIAGONAL slots (k < 8) use slot-ALIGNED 2-subchunk exp groups so
    # their affine_select masks never wait on slot 2's scores (a
    # cross-slot triple here stalled PV(1) -> PE -> Act ~2.7us every
    # rep), and the upper groups (subchunks 2,3: fully masked for
    # i < 256) exp only columns 256.. . Slots 2..16 stream exactly 20
    # aligned 3-subchunk groups, each exp'd with ONE 1536-wide
    # activation (the ~376ns fixed cost per activation is the Act
    # engine's main tax above its 0.79ns/col streaming rate). pt tiles
    # mirror the group layout; PV picks subchunk slices via pt_loc.
    K_DIAG = 2 * NJS
    trip = {"ps": None, "pt": None}
    pt_loc = {}

    def sc_subchunk(t, s, x_t):
        k = NJS * t + s
        if k < K_DIAG:
            start = (k // 2) * 2
            r = k - start
            end = start + 1
            trim = 256 if (k % NJS) >= 2 else 0
        else:
            start = K_DIAG + ((k - K_DIAG) // 3) * 3
            r = k - start
            end = start + 2
            trim = 0
        if r == 0:
            ps_tr = ps_s_p.tile([JS, 3, CH], F32, tag="pss")
            pt_tr = ptp.tile([JS, 3, CH], BF16, tag="pt")
            trip["ps"], trip["pt"] = ps_tr, pt_tr
        nc.tensor.matmul(
            trip["ps"][:, r, :],
            x_t[:, 0, s * JS:(s + 1) * JS],
            x_t[:, 1, :],
            start=True,
            stop=True,
        )
        pt_loc[k] = (trip["pt"], r)
        if k == end:
            nc.scalar.activation(
                trip["pt"][:, 0:r + 1, trim:CH],
                trip["ps"][:, 0:r + 1, trim:CH],
                mybir.ActivationFunctionType.Exp,
            )
            # causal mask for diagonal-slot subchunks; the exp-skipped
            # columns (fully masked) are memset to 0 without a read, and
            # the affine predicate is rebased to the trimmed AP origin
            for kk in range(start, k + 1):
                if kk < K_DIAG:
                    ss = kk % NJS
                    ptile, rr = pt_loc[kk]
                    tr = 256 if ss >= 2 else 0
                    if tr:
                        nc.gpsimd.memset(ptile[:, rr, 0:tr], 0.0)
                    nc.gpsimd.affine_select(
                        out=ptile[:, rr, tr:CH],
                        in_=ptile[:, rr, tr:CH],
                        compare_op=mybir.AluOpType.is_ge,
                        fill=0.0,
                        base=-(ss * JS) + tr,
                        pattern=[[1, CH - tr]],
                        channel_multiplier=-1,
                    )

    def scores_part(t, x_t):
        for s in range(NJS):
            sc_subchunk(t, s, x_t)

    def pv_part(t, v_sb):
        # oT += v_aug^T @ pT  (row 64 accumulates the denominator)
        ps_o = ps_o_p.tile([DA, CH], F32, tag="pso")
        for s in range(NJS):
            ptile, r = pt_loc[NJS * t + s]
            nc.tensor.matmul(
                ps_o[:],
                v_sb[:, s * DA:(s + 1) * DA],
                ptile[:, r, :],
                start=(s == 0),
                stop=(s == NJS - 1),
            )
        return ps_o

    def accum(t, ps_o):
        # accA += gamma * partial, accB += (1-gamma) * partial. gamma is
        # only data-dependent for slots 2..8: slot 0 is always the A
        # diagonal (gamma=1), slot 1 the B diagonal and slots 9..16
        # B-pairs (gamma=0) on every core. For the data-dependent slots,
        # DVE builds g = gamma*ps_o and g2 = ps_o - g; the accumulator
        # adds run on GpSimd (SBUF-only operands) to keep DVE's serial
        # chain short. Static slots add straight on DVE.
        if t == 0:
            nc.vector.tensor_copy(accA[:], ps_o[:])
            return
        if t == 1:
            nc.vector.tensor_copy(accB[:], ps_o[:])
            return
        if 2 <= t <= 8:
            g = gdp.tile([DA, CH], F32, tag="g")
            g2 = gdp.tile([DA, CH], F32, tag="g2")
            nc.vector.tensor_scalar_mul(g[:], ps_o[:], gam[:, t:t + 1])
            nc.vector.tensor_sub(g2[:], ps_o[:], g[:])
            nc.gpsimd.tensor_add(accA[:], accA[:], g[:])
            nc.vector.tensor_add(accB[:], accB[:], g2[:])
        else:
            nc.vector.tensor_add(accB[:], accB[:], ps_o[:])

    def finalize(pair, acc, only_s=None):
        # normalize + transpose back + store one 512-row chunk
        o = finalize.o.get(pair) if only_s else None
        if o is None:
            o = finp.tile([JS, NJS, D], F32, tag="o")
            finalize.o[pair] = o
        for s in (range(NJS) if only_s is None else only_s):
            ps_t = ps_o_p.tile([JS, DA], F32, tag="pso")
            nc.tensor.transpose(
                ps_t[:], acc[:, s * JS:(s + 1) * JS], ident[:]
            )
            r = finp.tile([JS, 1], F32, tag="r")
            nc.vector.reciprocal(r[:], ps_t[:, D:DA])
            nc.vector.tensor_scalar_mul(o[:, s, :], ps_t[:, 0:D], r[:])
        if only_s is None or only_s[-1] == NJS - 1:
            nc.sync.dma_start(
                out=out_d[pair, :, :].rearrange("(s p) d -> p s d", p=JS),
                in_=o[:],
            )
    finalize.o = {}

    # ---- slot loop (software-pipelined, PV lags by one slot) ----
    # Emission order per iteration: scores(t) -> v(t) -> dma(t+1) ->
    # PV(t-1), so exp(0) starts as early as possible (scores head the PE
    # stream) and the in-order PE stream always has the next slot's
    # exp-independent matmuls queued while the Act engine drains exp(t);
    # steady state is Act-bound with PE ~87% occupied.
    x_t = xbufs[0] if x0_ready else dma_x(0)
    v_sb = dma_v(0)
    prev = None            # (pt, v_sb) awaiting PV
    pending = None         # ps_o awaiting accumulate
    for t in range(NSLOT):
        scores_part(t, x_t)
        cur_v = v_sb
        if t + 1 < NSLOT:
            x_t = dma_x(t + 1)
            v_sb = dma_v(t + 1)
        if prev is not None:
            ps_o = pv_part(t - 1, prev)
            if pending is not None:
                accum(t - 2, pending)
            pending = ps_o
        if OVERLAP_FINALS and t in (10, 12, 14, 16):
            # every core's A-chunk slots are within slots 0..8, so accA
            # is final once accum(8) ran (during the t=10 iteration);
            # spread its output pass one 128-row subtile per slot
            finalize(0, accA, only_s=[(t - 10) // 2])
        prev = cur_v
    # drain the pipeline: PV + accum for the last two slots. The final
    # slot's partial folds into accB subtile-by-subtile so PE transposes,
    # DVE normalizes and the store DMA pipeline against each other.
    ps_o = pv_part(NSLOT - 1, prev)
    accum(NSLOT - 2, pending)
    o1 = finp.tile([JS, NJS, D], F32, tag="o")
    for s in range(NJS):
        sl = slice(s * JS, (s + 1) * JS)
        nc.vector.tensor_add(accB[:, sl], accB[:, sl], ps_o[:, sl])
        ps_t = ps_o_p.tile([JS, DA], F32, tag="pso")
        nc.tensor.transpose(ps_t[:], accB[:, sl], ident[:])
        r = finp.tile([JS, 1], F32, tag="r")
        nc.vector.reciprocal(r[:], ps_t[:, D:DA])
        nc.vector.tensor_scalar_mul(o1[:, s, :], ps_t[:, 0:D], r[:])
    nc.sync.dma_start(
        out=out_d[1, :, :].rearrange("(s p) d -> p s d", p=JS),
        in_=o1[:],
    )

    if not OVERLAP_FINALS:
        finalize(0, accA)


_NC_CACHE = None


def _get_program():
    global _NC_CACHE
    if _NC_CACHE is None:
        _NC_CACHE = _build_program()
    return _NC_CACHE


def _host_inputs(x, w_q, b_q, w_k, b_k, w_v, b_v):
    """Per-core input dicts. Host work: layout (transpose/gather/concat of
    x rows), weight reshuffles, and one tiny 65x65 projection of the query
    rows (qtil), all O(S*d^2) -- negligible next to the S^2 device work."""
    import ml_dtypes

    BF = ml_dtypes.bfloat16
    x = np.ascontiguousarray(np.asarray(x, dtype=np.float32))
    scale = 1.0 / np.sqrt(np.float32(D))

    wk_aug = np.concatenate([np.asarray(w_k, np.float32).T,
                             np.asarray(b_k, np.float32)[None, :]], axis=0)
    wq_aug = np.concatenate([np.asarray(w_q, np.float32).T,
                             np.asarray(b_q, np.float32)[None, :]], axis=0) * scale
    wv_aug = np.zeros((DA, DA), np.float32)
    wv_aug[:D, :D] = np.asarray(w_v, np.float32).T
    wv_aug[D, :D] = np.asarray(b_v, np.float32)
    wv_aug[D, D] = 1.0
    ident = np.eye(DA, dtype=np.float32)

    xT_aug = np.empty((DA, S), np.float32)
    xT_aug[:D] = x.T
    xT_aug[D] = 1.0
    # scores[i, j] = xq_aug_i^T (wq_aug wk_aug^T) xk_aug_j = xk_aug_j . qtil_i
    # with qtil_i = (wk_aug wq_aug^T) xq_aug_i
    m2 = wk_aug @ wq_aug.T
    qtilT = (m2 @ xT_aug).astype(BF)
    xT16 = xT_aug.astype(BF)
    # v_aug rows (col 64 = ones), packed per 512-row block in the PV
    # stationary layout [j (128), subchunk s (4), da (65)]
    vfull = (xT_aug.T @ wv_aug).astype(BF)          # [S, DA]
    v_pan = vfull.reshape(NCH, NJS, JS, DA).transpose(0, 2, 1, 3)

    CPW = NSLOT + DA
    in_maps = []
    for m in range(N_CORES):
        slots, gam = _schedule(m)
        xall = np.empty((DA, NSLOT, 2, CH), BF)
        vall = np.empty((JS, NSLOT, NJS, DA), BF)
        for t, (b, c) in enumerate(slots):
            xall[:, t, 0, :] = xT16[:, b * CH:(b + 1) * CH]
            xall[:, t, 1, :] = qtilT[:, c * CH:(c + 1) * CH]
            vall[:, t] = v_pan[b]
        cpack = np.zeros((DA, CPW), np.float32)
        cpack[:, 0:NSLOT] = np.asarray(gam, np.float32)[None, :]
        cpack[:, NSLOT:CPW] = ident
        in_maps.append({
            "xall": xall.reshape(DA, NSLOT * 2 * CH),
            "vall": vall.reshape(JS, NSLOT * NJS * DA),
            "cpack": cpack,
        })
    return in_maps


def _assemble(results):
    out = np.empty((S, D), np.float32)
    for m in range(N_CORES):
        op = results[m]["out_pair"]
        A, B = m, NCH - 1 - m
        out[A * CH:(A + 1) * CH] = op[0]
        out[B * CH:(B + 1) * CH] = op[1]
    return out


def kernel(x, w_q, b_q, w_k, b_k, w_v, b_v):
    nc = _get_program()
    in_maps = _host_inputs(x, w_q, b_q, w_k, b_k, w_v, b_v)
    res = run_bass_kernel_spmd(nc, in_maps, list(range(N_CORES)))
    return _assemble(res.results)


# revision 47
# speedup vs baseline: 1.1906x; 1.1906x over previous
"""Causal single-head attention (S=8192, d=64) on 8 Trainium2 NeuronCores.

Strategy (sequence-parallel, load-balanced over the causal triangle):
  - Split the sequence into 16 chunks of 512 rows. Core m owns query chunks
    A=m and B=15-m, so every core sees (m+1) + (16-m) = 17 (kv-block, q-chunk)
    pairs of 512x512 scores -- perfectly balanced.
  - Each pair is one "slot". The SPMD program is identical on all cores; the
    per-core schedule is baked into the *input data*: the host sends, per
    slot, the kv rows (transposed + bias-augmented) AND the matching
    projected-query panel qtil = (wq_aug wk_aug^T) @ xq_aug^T, so the device
    runs zero per-slot projection matmuls for the score side.
  - Scores are computed transposed, sT[j, i] (kv j on partitions):
      sT = xkvT[:, js]^T @ qtil   (stationary straight from the DMA)
    so the softmax denominator comes from an extra all-ones column of the
    augmented V in the PV matmul, and P^T feeds the PV matmul directly.
  - All matmul operands are bf16 (the correctness gate is 2e-2; bf16 puts
    global rel err ~1e-3): 4x faster v-projection matmuls, half the DMA and
    SBUF traffic, and bf16 pt/v_sb feed PE at 1 cycle/row.
  - The causal mask is needed only on the two diagonal slots (slot 0 and 1)
    and is applied as a post-exp affine_select (zero the j > i half).
  - Per-slot partials accumulate into per-chunk SBUF accumulators via a 0/1
    multiplier input (gamma), keeping the program fully static.
  - PV(t) is emitted AFTER scores(t+1) so the in-order PE stream always has
    exp-independent work while the Act engine (the busiest engine, ~35us of
    exp) drains; steady state is Act-bound.

Math per slot t with kv block rows Xk = x[512b:512b+512], q rows Xq:
  xkvT = [Xk^T; 1]  (65 x 512)     qtil = M2 @ [Xq^T; 1]  (65 x 512, host)
  M2   = wq_aug @ wk_aug^T / 8     (65 x 65, host-precomputed from weights)
  sT   = xkvT[:, js]^T @ qtil      (128 x 512 per 128-row j-subchunk, PSUM)
  pT   = exp(sT)                   (masked to causal on diagonal slots)
  v    = xkvT[:, js]^T @ wv_aug    (128 x 65 per j-subchunk; col 64 = ones)
  oT  += v[js]^T @ pT              (65 x 512; row 64 = softmax denominator)
Final: out[i, :] = oT[0:64, i] / oT[64, i], transposed back via PE transpose.
"""

import sys

sys.path.insert(0, "/opt/trn_rl_repo")

import numpy as np
import concourse.bass as bass
import concourse.mybir as mybir
from concourse import tile
from concourse.bass_utils import run_bass_kernel_spmd

OVERLAP_FINALS = True
N_CORES = 8
S = 8192
D = 64
CH = 512
NCH = S // CH          # 16 chunks
NSLOT = 17             # (kv, q) pairs per core
JS = 128               # j-subchunk (PSUM partition dim)
NJS = CH // JS         # 4
DA = D + 1             # bias-augmented contraction dim

F32 = mybir.dt.float32
BF16 = mybir.dt.bfloat16


def _split_multiwait(nc, max_waits=1):
    """The walrus build in this container accepts only one sync-wait per
    instruction; hoist extra waits onto preceding same-engine NOPs."""
    for func in nc.m.functions:
        for bb in func.blocks:
            new_insts = []
            for inst in bb.instructions:
                si = inst.sync_info
                if si is not None and si.on_wait and len(si.on_wait) > max_waits:
                    waits = list(si.on_wait)
                    rest, head = waits[:-max_waits], waits[-max_waits:]
                    for j, w in enumerate(rest):
                        nop = mybir.InstNoOp(
                            name=f"{inst.name}-wsplit{j}", ins=[], outs=[]
                        )
                        nop.engine = inst.engine
                        nop.sync_info = mybir.SyncInfo(on_wait=[w], on_update=[])
                        new_insts.append(nop)
                    inst.sync_info = mybir.SyncInfo(
                        on_wait=head, on_update=si.on_update
                    )
                new_insts.append(inst)
            bb.instructions = new_insts


def _schedule(m):
    """Slot list [(kv_block, q_chunk)] for core m; diagonal pairs first."""
    A, B = m, NCH - 1 - m
    slots = [(A, A), (B, B)]
    slots += [(b, A) for b in range(A)]
    slots += [(b, B) for b in range(B)]
    gam = [1.0, 0.0] + [1.0] * A + [0.0] * B
    assert len(slots) == NSLOT
    return slots, gam


def _build_program(repeat=1, dynamic=False):
    nc = bass.Bass()

    # xall[:, t, 0, :] = slot t's kv rows transposed+augmented (bf16);
    # [:, t, 1, :] = its host-projected qtil panel. One DMA per slot.
    xall_d = nc.declare_dram_parameter(
        "xall", [DA, NSLOT * 2 * CH], BF16, isOutput=False
    )
    # vall[:, t, s, :] = slot t's host-projected v_aug rows for j-subchunk s
    # (col 64 = ones -> softmax denominator row of the PV output)
    vall_d = nc.declare_dram_parameter(
        "vall", [JS, NSLOT * NJS * DA], BF16, isOutput=False
    )
    # packed fp32 constants: [gamma | identity]
    CPW = NSLOT + DA
    cpack_d = nc.declare_dram_parameter("cpack", [DA, CPW], F32, isOutput=False)
    out_d = nc.declare_dram_parameter("out_pair", [2, CH, D], F32, isOutput=True)

    NXBUF = 6
    with tile.TileContext(nc) as tc:
        with (
            tc.tile_pool(name="consts", bufs=1) as consts,
            tc.tile_pool(name="slot_in", bufs=NXBUF) as slot_in,
            tc.tile_pool(name="slot_mid", bufs=6) as slot_mid,
            tc.tile_pool(name="acc", bufs=1) as accp,
            tc.tile_pool(name="pt", bufs=6) as ptp,
            tc.tile_pool(name="gd", bufs=6) as gdp,
            tc.tile_pool(name="fin", bufs=4) as finp,
            tc.tile_pool(name="ps_s", bufs=2, space="PSUM") as ps_s_p,
            tc.tile_pool(name="ps_o", bufs=2, space="PSUM") as ps_o_p,
        ):
            # ---- x tiles are [128, 2, CH] with only rows 0..64 DMA'd per
            # slot. Rows 65..127 are zeroed ONCE per ring buffer here (and
            # never written again), so the scores matmuls can run with a
            # zero-padded 128-row contraction: the PE streams moving rows
            # at 1 row/cycle only at full 128-partition contraction
            # (measured ~0.41 ns/row vs ~0.83 at 65) -- the padded zeros
            # buy a ~1.76x faster scores stage. The padded qt rows must
            # also be finite (0 * garbage = NaN if garbage is inf/nan), so
            # both halves get the one-time memset.
            xbufs = []
            for b in range(NXBUF):
                x_b = slot_in.tile([JS, 2, CH], BF16, tag="x_t")
                # partition starts must be 0/32/64/96-aligned: zero rows
                # 64..127 (the DMA rewrites row 64 -- the aug row -- each
                # slot; rows 65..127 stay zero for the padded contraction)
                nc.gpsimd.memset(x_b[D:JS, :, :], 0.0)
                xbufs.append(x_b)
            # first slot's x DMA heads the critical path (scores of slot 0)
            x_t0 = xbufs[0]
            nc.sync.dma_start(
                out=x_t0[0:DA, :, :],
                in_=xall_d[:, 0:2 * CH].rearrange("p (h c) -> p h c", h=2),
            )
            cpack = consts.tile([DA, CPW], F32)
            nc.sync.dma_start(out=cpack[:], in_=cpack_d[:])
            gam = cpack[:, 0:NSLOT]
            ident = cpack[:, NSLOT:CPW]

            # ---- body (repeat>1 builds a timing-calibration NEFF) ----
            if dynamic and repeat > 1:
                with tc.For_i(0, repeat, 1):
                    _body(nc, tc, xbufs, slot_mid, ptp, gdp, finp,
                          ps_s_p, ps_o_p, accp,
                          xall_d, vall_d, out_d, gam, ident, False)
            else:
                for _rep in range(repeat):
                    _body(nc, tc, xbufs, slot_mid, ptp, gdp, finp,
                          ps_s_p, ps_o_p, accp,
                          xall_d, vall_d, out_d, gam, ident, _rep == 0)

    _split_multiwait(nc)
    return nc


def _body(nc, tc, xbufs, slot_mid, ptp, gdp, finp,
          ps_s_p, ps_o_p, accp,
          xall_d, vall_d, out_d, gam, ident, x0_ready):
    # ---- per-chunk accumulators ----
    accA = accp.tile([DA, CH], F32, tag="accA")
    accB = accp.tile([DA, CH], F32, tag="accB")

    def dma_x(t):
        # manual ring over the memset-prepped buffers: rows 65..127 stay
        # zero (the scores matmuls' padded contraction), DMA only 0..64
        x_t = xbufs[t % len(xbufs)]
        nc.sync.dma_start(
            out=x_t[0:DA, :, :],
            in_=xall_d[:, t * 2 * CH:(t + 1) * 2 * CH].rearrange(
                "p (h c) -> p h c", h=2
            ),
        )
        return x_t

    def dma_v(t):
        v_sb = slot_mid.tile([JS, NJS * DA], BF16, tag="v_sb")
        nc.sync.dma_start(
            out=v_sb[:], in_=vall_d[:, t * NJS * DA:(t + 1) * NJS * DA]
        )
        return v_sb

    # ---- subchunk-stream score/exp tiling (hybrid) ----
    # The j-subchunks form one stream k = 4t+s (68 total). The two
    # DIAGONAL slots (k < 8) use slot-ALIGNED 2-subchunk exp groups so
    # their affine_select masks never wait on slot 2's scores (a
    # cross-slot triple here stalled PV(1) -> PE -> Act ~2.7us every
    # rep), and the upper groups (subchunks 2,3: fully masked for
    # i < 256) exp only columns 256.. . Slots 2..16 stream exactly 20
    # aligned 3-subchunk groups, each exp'd with ONE 1536-wide
    # activation (the ~376ns fixed cost per activation is the Act
    # engine's main tax above its 0.79ns/col streaming rate). pt tiles
    # mirror the group layout; PV picks subchunk slices via pt_loc.
    K_DIAG = 2 * NJS
    trip = {"ps": None, "pt": None}
    pt_loc = {}

    def sc_subchunk(t, s, x_t):
        k = NJS * t + s
        if k < K_DIAG:
            start = (k // 2) * 2
            r = k - start
            end = start + 1
            trim = 256 if (k % NJS) >= 2 else 0
        else:
            start = K_DIAG + ((k - K_DIAG) // 3) * 3
            r = k - start
            end = start + 2
            trim = 0
        if r == 0:
            ps_tr = ps_s_p.tile([JS, 3, CH], F32, tag="pss")
            pt_tr = ptp.tile([JS, 3, CH], BF16, tag="pt")
            trip["ps"], trip["pt"] = ps_tr, pt_tr
        nc.tensor.matmul(
            trip["ps"][:, r, :],
            x_t[:, 0, s * JS:(s + 1) * JS],
            x_t[:, 1, :],
            start=True,
            stop=True,
        )
        pt_loc[k] = (trip["pt"], r)
        if k == end:
            nc.scalar.activation(
                trip["pt"][:, 0:r + 1, trim:CH],
                trip["ps"][:, 0:r + 1, trim:CH],
                mybir.ActivationFunctionType.Exp,
            )
            # causal mask for diagonal-slot subchunks; the exp-skipped
            # columns (fully masked) are memset to 0 without a read, and
            # the affine predicate is rebased to the trimmed AP origin
            for kk in range(start, k + 1):
                if kk < K_DIAG:
                    ss = kk % NJS
                    ptile, rr = pt_loc[kk]
                    tr = 256 if ss >= 2 else 0
                    if tr:
                        nc.vector.memset(ptile[:, rr, 0:tr], 0.0)
                    nc.gpsimd.affine_select(
                        out=ptile[:, rr, tr:CH],
                        in_=ptile[:, rr, tr:CH],
                        compare_op=mybir.AluOpType.is_ge,
                        fill=0.0,
                        base=-(ss * JS) + tr,
                        pattern=[[1, CH - tr]],
                        channel_multiplier=-1,
                    )

    def scores_part(t, x_t):
        for s in range(NJS):
            sc_subchunk(t, s, x_t)

    def pv_part(t, v_sb):
        # oT += v_aug^T @ pT  (row 64 accumulates the denominator)
        ps_o = ps_o_p.tile([DA, CH], F32, tag="pso")
        for s in range(NJS):
            ptile, r = pt_loc[NJS * t + s]
            nc.tensor.matmul(
                ps_o[:],
                v_sb[:, s * DA:(s + 1) * DA],
                ptile[:, r, :],
                start=(s == 0),
                stop=(s == NJS - 1),
            )
        return ps_o

    def accum(t, ps_o):
        # accA += gamma * partial, accB += (1-gamma) * partial. gamma is
        # only data-dependent for slots 2..8: slot 0 is always the A
        # diagonal (gamma=1), slot 1 the B diagonal and slots 9..16
        # B-pairs (gamma=0) on every core. For the data-dependent slots,
        # DVE builds g = gamma*ps_o and g2 = ps_o - g; the accumulator
        # adds run on GpSimd (SBUF-only operands) to keep DVE's serial
        # chain short. Static slots add straight on DVE.
        if t == 0:
            nc.vector.tensor_copy(accA[:], ps_o[:])
            return
        if t == 1:
            nc.vector.tensor_copy(accB[:], ps_o[:])
            return
        if 2 <= t <= 8:
            g = gdp.tile([DA, CH], F32, tag="g")
            g2 = gdp.tile([DA, CH], F32, tag="g2")
            nc.vector.tensor_scalar_mul(g[:], ps_o[:], gam[:, t:t + 1])
            nc.vector.tensor_sub(g2[:], ps_o[:], g[:])
            nc.gpsimd.tensor_add(accA[:], accA[:], g[:])
            nc.vector.tensor_add(accB[:], accB[:], g2[:])
        else:
            nc.vector.tensor_add(accB[:], accB[:], ps_o[:])

    def finalize(pair, acc, only_s=None):
        # normalize + transpose back + store one 512-row chunk
        o = finalize.o.get(pair) if only_s else None
        if o is None:
            o = finp.tile([JS, NJS, D], F32, tag="o")
            finalize.o[pair] = o
        for s in (range(NJS) if only_s is None else only_s):
            ps_t = ps_o_p.tile([JS, DA], F32, tag="pso")
            nc.tensor.transpose(
                ps_t[:], acc[:, s * JS:(s + 1) * JS], ident[:]
            )
            r = finp.tile([JS, 1], F32, tag="r")
            nc.vector.reciprocal(r[:], ps_t[:, D:DA])
            nc.vector.tensor_scalar_mul(o[:, s, :], ps_t[:, 0:D], r[:])
        if only_s is None or only_s[-1] == NJS - 1:
            nc.sync.dma_start(
                out=out_d[pair, :, :].rearrange("(s p) d -> p s d", p=JS),
                in_=o[:],
            )
    finalize.o = {}

    # ---- slot loop (software-pipelined, PV lags by one slot) ----
    # Emission order per iteration: scores(t) -> v(t) -> dma(t+1) ->
    # PV(t-1), so exp(0) starts as early as possible (scores head the PE
    # stream) and the in-order PE stream always has the next slot's
    # exp-independent matmuls queued while the Act engine drains exp(t);
    # steady state is Act-bound with PE ~87% occupied.
    x_t = xbufs[0] if x0_ready else dma_x(0)
    v_sb = dma_v(0)
    vq = []                # v panels awaiting PV (lag 2)
    pending = None         # ps_o awaiting accumulate
    for t in range(NSLOT):
        scores_part(t, x_t)
        vq.append(v_sb)
        if t + 1 < NSLOT:
            x_t = dma_x(t + 1)
            v_sb = dma_v(t + 1)
        if t >= 2:
            # PV lags TWO slots: by the time the in-order PE stream
            # reaches PV(0)/PV(1), the diagonal slots' Pool affine chain
            # (~5us serial at rep start) has drained -- with lag 1 it
            # stalled PE -> Act ~2.6us every rep
            ps_o = pv_part(t - 2, vq.pop(0))
            if pending is not None:
                accum(t - 3, pending)
            pending = ps_o
        if OVERLAP_FINALS and t in (11, 13, 15, 16):
            # accA is final once accum(8) ran (earlier in the t=11
            # iteration); spread its output pass one subtile per window
            finalize(0, accA, only_s=[{11: 0, 13: 1, 15: 2, 16: 3}[t]])
    # drain the pipeline: PV + accum for the last slots; the final
    # slot's partial folds into accB subtile-by-subtile so PE
    # transposes, DVE normalizes and the store DMA pipeline.
    ps_o15 = pv_part(NSLOT - 2, vq.pop(0))
    accum(NSLOT - 3, pending)
    ps_o = pv_part(NSLOT - 1, vq.pop(0))
    accum(NSLOT - 2, ps_o15)
    o1 = finp.tile([JS, NJS, D], F32, tag="o")
    for s in range(NJS):
        sl = slice(s * JS, (s + 1) * JS)
        nc.vector.tensor_add(accB[:, sl], accB[:, sl], ps_o[:, sl])
        ps_t = ps_o_p.tile([JS, DA], F32, tag="pso")
        nc.tensor.transpose(ps_t[:], accB[:, sl], ident[:])
        r = finp.tile([JS, 1], F32, tag="r")
        nc.vector.reciprocal(r[:], ps_t[:, D:DA])
        nc.vector.tensor_scalar_mul(o1[:, s, :], ps_t[:, 0:D], r[:])
    nc.sync.dma_start(
        out=out_d[1, :, :].rearrange("(s p) d -> p s d", p=JS),
        in_=o1[:],
    )

    if not OVERLAP_FINALS:
        finalize(0, accA)


_NC_CACHE = None


def _get_program():
    global _NC_CACHE
    if _NC_CACHE is None:
        _NC_CACHE = _build_program()
    return _NC_CACHE


def _host_inputs(x, w_q, b_q, w_k, b_k, w_v, b_v):
    """Per-core input dicts. Host work: layout (transpose/gather/concat of
    x rows), weight reshuffles, and one tiny 65x65 projection of the query
    rows (qtil), all O(S*d^2) -- negligible next to the S^2 device work."""
    import ml_dtypes

    BF = ml_dtypes.bfloat16
    x = np.ascontiguousarray(np.asarray(x, dtype=np.float32))
    scale = 1.0 / np.sqrt(np.float32(D))

    wk_aug = np.concatenate([np.asarray(w_k, np.float32).T,
                             np.asarray(b_k, np.float32)[None, :]], axis=0)
    wq_aug = np.concatenate([np.asarray(w_q, np.float32).T,
                             np.asarray(b_q, np.float32)[None, :]], axis=0) * scale
    wv_aug = np.zeros((DA, DA), np.float32)
    wv_aug[:D, :D] = np.asarray(w_v, np.float32).T
    wv_aug[D, :D] = np.asarray(b_v, np.float32)
    wv_aug[D, D] = 1.0
    ident = np.eye(DA, dtype=np.float32)

    xT_aug = np.empty((DA, S), np.float32)
    xT_aug[:D] = x.T
    xT_aug[D] = 1.0
    # scores[i, j] = xq_aug_i^T (wq_aug wk_aug^T) xk_aug_j = xk_aug_j . qtil_i
    # with qtil_i = (wk_aug wq_aug^T) xq_aug_i
    m2 = wk_aug @ wq_aug.T
    qtilT = (m2 @ xT_aug).astype(BF)
    xT16 = xT_aug.astype(BF)
    # v_aug rows (col 64 = ones), packed per 512-row block in the PV
    # stationary layout [j (128), subchunk s (4), da (65)]
    vfull = (xT_aug.T @ wv_aug).astype(BF)          # [S, DA]
    v_pan = vfull.reshape(NCH, NJS, JS, DA).transpose(0, 2, 1, 3)

    CPW = NSLOT + DA
    in_maps = []
    for m in range(N_CORES):
        slots, gam = _schedule(m)
        xall = np.empty((DA, NSLOT, 2, CH), BF)
        vall = np.empty((JS, NSLOT, NJS, DA), BF)
        for t, (b, c) in enumerate(slots):
            xall[:, t, 0, :] = xT16[:, b * CH:(b + 1) * CH]
            xall[:, t, 1, :] = qtilT[:, c * CH:(c + 1) * CH]
            vall[:, t] = v_pan[b]
        cpack = np.zeros((DA, CPW), np.float32)
        cpack[:, 0:NSLOT] = np.asarray(gam, np.float32)[None, :]
        cpack[:, NSLOT:CPW] = ident
        in_maps.append({
            "xall": xall.reshape(DA, NSLOT * 2 * CH),
            "vall": vall.reshape(JS, NSLOT * NJS * DA),
            "cpack": cpack,
        })
    return in_maps


def _assemble(results):
    out = np.empty((S, D), np.float32)
    for m in range(N_CORES):
        op = results[m]["out_pair"]
        A, B = m, NCH - 1 - m
        out[A * CH:(A + 1) * CH] = op[0]
        out[B * CH:(B + 1) * CH] = op[1]
    return out


def kernel(x, w_q, b_q, w_k, b_k, w_v, b_v):
    nc = _get_program()
    in_maps = _host_inputs(x, w_q, b_q, w_k, b_k, w_v, b_v)
    res = run_bass_kernel_spmd(nc, in_maps, list(range(N_CORES)))
    return _assemble(res.results)
